# revision 3
# baseline (speedup 1.0000x reference)
"""CrossCompressUnit TRN2 kernel.

v_out = v * (e.w_vv) + e * (v.w_ev) + bias_v
e_out = v * (e.w_ve) + e * (v.w_ee) + bias_e

Data-parallel over batch across 8 NeuronCores (2048 rows/core).
Per core, per 128-row block ("granule"):
  - products m_k = {e,v} * w_k on VectorE (tensor_tensor, weights broadcast
    across partitions once at start)
  - per-row dots = free-dim reduce of m_k on ScalarE (activation accum_out)
  - per-row 2x2 mixing via diagonal matmuls on the TensorEngine accumulating
    in PSUM (diag = identity * per-partition scalar via VectorE tensor_scalar)
  - PSUM evacuated to SBUF split across ScalarE/VectorE, stored via HWDGE DMA
"""

import numpy as np
from contextlib import ExitStack

import concourse.bass as bass
import concourse.bacc as bacc
import concourse.tile as tile
from concourse import mybir
from concourse import bass_utils

NCORES = 8
B = 16384
D = 1024
RPC = B // NCORES          # rows per core
P = 128                    # partitions
NBLK = RPC // P            # 16 row-blocks per core
NPG = 2                    # row-blocks per supertile (1MB DMAs)
NST = NBLK // NPG          # supertiles per core

F32 = mybir.dt.float32

_built = {}
LAST_RESULT = None
TRACE = False


def _build(with_bias: bool):
    nc = bacc.Bacc(
        "TRN2",
        target_bir_lowering=False,
        debug=False,
        enable_asserts=False,
        num_devices=NCORES,
    )

    v_d = nc.dram_tensor("v", [RPC, D], F32, kind="ExternalInput").ap()
    e_d = nc.dram_tensor("e", [RPC, D], F32, kind="ExternalInput").ap()
    w_d = nc.dram_tensor("wcat", [4 * D], F32, kind="ExternalInput").ap()
    id_d = nc.dram_tensor("ident", [P, P], F32, kind="ExternalInput").ap()
    if with_bias:
        b_d = nc.dram_tensor("bcat", [1, 2 * D], F32, kind="ExternalInput").ap()
    vo_d = nc.dram_tensor("vout", [RPC, D], F32, kind="ExternalOutput").ap()
    eo_d = nc.dram_tensor("eout", [RPC, D], F32, kind="ExternalOutput").ap()

    # [128, 16, 1024]: partition = row within block, middle = row-block
    vr = v_d.rearrange("(n p) d -> p n d", p=P)
    er = e_d.rearrange("(n p) d -> p n d", p=P)
    vor = vo_d.rearrange("(n p) d -> p n d", p=P)
    eor = eo_d.rearrange("(n p) d -> p n d", p=P)

    MULT = mybir.AluOpType.mult
    COPY = mybir.ActivationFunctionType.Copy

    with tile.TileContext(nc) as tc:
        with ExitStack() as ctx:
            singles = ctx.enter_context(tc.tile_pool(name="singles", bufs=1))
            io_pool = ctx.enter_context(tc.tile_pool(name="io", bufs=3))
            m_pool = ctx.enter_context(tc.tile_pool(name="m", bufs=2))
            dg_pool = ctx.enter_context(tc.tile_pool(name="diag", bufs=3))
            sm_pool = ctx.enter_context(tc.tile_pool(name="small", bufs=4))
            ps_pool = ctx.enter_context(
                tc.tile_pool(name="psum", bufs=2, space="PSUM")
            )

            # Broadcast all 4 weight vectors across partitions: [128, 4096]
            wb = singles.tile([P, 4 * D], F32)
            w_bcast = bass.AP(
                tensor=w_d.tensor, offset=w_d.offset, ap=[[0, P], w_d.ap[0]]
            )
            nc.gpsimd.dma_start(out=wb, in_=w_bcast)

            ident = singles.tile([P, P], F32)
            nc.sync.dma_start(out=ident, in_=id_d)

            # scratch sink for the ACT-reduce primary outputs
            garbage = singles.tile([P, D], F32)

            if with_bias:
                ones1 = singles.tile([1, P], F32)
                nc.vector.memset(ones1, 1.0)
                brow = singles.tile([1, 2 * D], F32)
                nc.sync.dma_start(out=brow, in_=b_d)

            for t in range(NST):
                blk = slice(t * NPG, (t + 1) * NPG)
                vt = io_pool.tile([P, NPG, D], F32, tag="vt")
                et = io_pool.tile([P, NPG, D], F32, tag="et")
                nc.sync.dma_start(out=vt, in_=vr[:, blk, :])
                nc.sync.dma_start(out=et, in_=er[:, blk, :])
                vo = io_pool.tile([P, NPG, D], F32, tag="vo")
                eo = io_pool.tile([P, NPG, D], F32, tag="eo")

                for g in range(NPG):
                    vg = vt[:, g, :]
                    eg = et[:, g, :]

                    # m_k = src_k * w_k  (VectorE)
                    # k: 0 = e.w_vv, 1 = v.w_ev, 2 = e.w_ve, 3 = v.w_ee
                    m4 = m_pool.tile([P, 4, D], F32, tag="m4")
                    for k, src in enumerate((eg, vg, eg, vg)):
                        nc.vector.tensor_tensor(
                            out=m4[:, k, :],
                            in0=src,
                            in1=wb[:, k * D : (k + 1) * D],
                            op=MULT,
                        )

                    # per-row dots: free-dim sums on ScalarE
                    s = sm_pool.tile([P, 4], F32, tag="dots")
                    for k in range(4):
                        nc.scalar.activation(
                            out=garbage,
                            in_=m4[:, k, :],
                            func=COPY,
                            accum_out=s[:, k : k + 1],
                        )

                    # diag_k = diag(s_k) (VectorE tensor_scalar on identity)
                    dgs = dg_pool.tile([P, 4, P], F32, tag="dg")
                    for k in range(4):
                        nc.vector.tensor_scalar_mul(
                            dgs[:, k, :], ident, s[:, k : k + 1]
                        )

                    vps = ps_pool.tile([P, D], F32, tag="vps")
                    eps = ps_pool.tile([P, D], F32, tag="eps")
                    H = 512
                    for h in range(D // H):
                        sl = slice(h * H, (h + 1) * H)
                        nc.tensor.matmul(
                            vps[:, sl], dgs[:, 0, :], vg[:, sl],
                            start=True, stop=False,
                        )
                        nc.tensor.matmul(
                            vps[:, sl], dgs[:, 1, :], eg[:, sl],
                            start=False, stop=not with_bias,
                        )
                        nc.tensor.matmul(
                            eps[:, sl], dgs[:, 2, :], vg[:, sl],
                            start=True, stop=False,
                        )
                        nc.tensor.matmul(
                            eps[:, sl], dgs[:, 3, :], eg[:, sl],
                            start=False, stop=not with_bias,
                        )
                        if with_bias:
                            nc.tensor.matmul(
                                vps[:, sl], ones1, brow[0:1, sl],
                                start=False, stop=True,
                            )
                            nc.tensor.matmul(
                                eps[:, sl], ones1,
                                brow[0:1, D + h * H : D + (h + 1) * H],
                                start=False, stop=True,
                            )

                    # evacuate PSUM -> SBUF, split ScalarE / VectorE
                    nc.scalar.copy(out=vo[:, g, 0:H], in_=vps[:, 0:H])
                    nc.vector.tensor_copy(out=vo[:, g, H:D], in_=vps[:, H:D])
                    nc.scalar.copy(out=eo[:, g, :], in_=eps)

                nc.sync.dma_start(out=vor[:, blk, :], in_=vo)
                nc.sync.dma_start(out=eor[:, blk, :], in_=eo)

    nc.compile()
    return nc


def _get(with_bias: bool):
    if with_bias not in _built:
        _built[with_bias] = _build(with_bias)
    return _built[with_bias]


def kernel(v, e, weight_vv, weight_ev, weight_ve, weight_ee, bias_v, bias_e):
    global LAST_RESULT
    v = np.ascontiguousarray(np.asarray(v, dtype=np.float32))
    e = np.ascontiguousarray(np.asarray(e, dtype=np.float32))
    bias_v = np.asarray(bias_v, dtype=np.float32)
    bias_e = np.asarray(bias_e, dtype=np.float32)
    with_bias = bool(np.any(bias_v) or np.any(bias_e))

    nc = _get(with_bias)

    wcat = np.concatenate(
        [
            np.asarray(w, dtype=np.float32).reshape(-1)
            for w in (weight_vv, weight_ev, weight_ve, weight_ee)
        ]
    )
    ident = np.eye(P, dtype=np.float32)
    bcat = np.concatenate([bias_v.reshape(-1), bias_e.reshape(-1)]).reshape(1, -1)

    in_maps = []
    for c in range(NCORES):
        rows = slice(c * RPC, (c + 1) * RPC)
        m = {"v": v[rows], "e": e[rows], "wcat": wcat, "ident": ident}
        if with_bias:
            m["bcat"] = bcat
        in_maps.append(m)

    res = bass_utils.run_bass_kernel_spmd(
        nc, in_maps, core_ids=list(range(NCORES)), trace=TRACE
    )
    LAST_RESULT = res

    vout = np.concatenate([r["vout"] for r in res.results], axis=0)
    eout = np.concatenate([r["eout"] for r in res.results], axis=0)
    return (vout, eout)
